# revision 14
# baseline (speedup 1.0000x reference)
"""CutOut kernel for Trainium2 (raw Bass), batch-sharded across 8 NeuronCores.

out[b,h,w,c] = 0 where (h,w) falls in the 50x50 rectangle centered at
(center_h[b], center_w[b]), else images[b,h,w,c]; labels pass through.

Strategy: data parallel over batch (8 samples/core, 24 MiB in + 24 MiB out
per core; the SBUF AXI fabric at ~436 GB/s is the binding resource ->
~110 us floor/core).  Written in raw Bass with explicit semaphores: the
walrus build in this environment accepts at most one inline sync-wait per
instruction, which rules out Tile's auto-generated multi-wait sync
(including its epilogue Drain) - standalone wait_ge instructions are the
legal form here.

Per core:
  consts: on the ACT HWDGE ring (idle at start, so image DMAs on the SP
          ring begin immediately): cmfull [1, 128+8*1536] (128 ones then
          per-sample column keep-masks) and flags [128, 32] ("row outside
          cutout?" 0/1 per row, laid out so tile i / subrow q reads column
          4i+q as a per-partition scalar).
  init:   24 K=1 matmuls (ones-weights outer product) broadcast the column
          masks to a [128, 12288] SBUF constant via PSUM; each sample's 3
          chunks are copied out right before that sample's compute, so the
          broadcast pipelines with the main loop instead of serializing.
  loop:   one 3 MiB DMA per sample (512 rows packed 4-per-partition,
          24 KiB contiguous per partition), four fused DVE ops
              out = (colmask max rowflag[p]) * img
          (exact 0/1 values -> bit-exact), one 3 MiB DMA out on ACT's ring.

Host prepares only O(B*(H+W)) flag values from the centers; all O(B*H*W)
work happens on device.
"""

import os
import sys
from contextlib import ExitStack

import numpy as np

if "/opt/trn_rl_repo" not in sys.path:
    sys.path.insert(0, "/opt/trn_rl_repo")

import concourse.bass as bass
import concourse.mybir as mybir
from concourse.bass_utils import run_bass_kernel_spmd

B, H, W, C = 64, 512, 512, 3
N_CORES = 8
BPC = B // N_CORES                 # samples per core = tiles per core
HALF = 25                          # 50x50 patch
ROWS = BPC * H                     # 4096 rows per core
COLS = W * C                       # 1536 floats per row
RPP = 4                            # rows packed per partition (512/128)
P = 128                            # SBUF partitions
CHUNK = 512                        # PSUM bank width (f32)
CMW = BPC * COLS                   # 12288 column-mask width
SLOTS = 4                          # in-flight sample tiles

F32 = mybir.dt.float32

_PROGRAM = None
LAST_RESULTS = None                # BassKernelResults of the latest run


def _build_program():
    nc = bass.Bass("TRN2", target_bir_lowering=False, debug=False,
                   num_devices=N_CORES)
    img = nc.dram_tensor("img", [ROWS, COLS], F32, kind="ExternalInput").ap()
    flags = nc.dram_tensor("flags", [P, RPP * BPC], F32,
                           kind="ExternalInput").ap()
    cmfull = nc.dram_tensor("cmfull", [1, P + CMW], F32,
                            kind="ExternalInput").ap()
    out = nc.dram_tensor("out", [ROWS, COLS], F32, kind="ExternalOutput").ap()

    with ExitStack() as ctx:
        slots = [ctx.enter_context(
                     nc.sbuf_tensor(f"slot{k}", [P, RPP * COLS], F32))
                 for k in range(SLOTS)]
        flags_sb = ctx.enter_context(
            nc.sbuf_tensor("flags_sb", [P, RPP * BPC], F32))
        cm_sb = ctx.enter_context(nc.sbuf_tensor("cm_sb", [1, P + CMW], F32))
        cmask = ctx.enter_context(nc.sbuf_tensor("cmask", [P, CMW], F32))
        psums = [ctx.enter_context(nc.psum_tensor(f"ps{k}", [P, CHUNK], F32))
                 for k in range(4)]
        const_sem = ctx.enter_context(nc.semaphore("const_sem"))
        pe_sem = ctx.enter_context(nc.semaphore("pe_sem"))
        cp_sem = ctx.enter_context(nc.semaphore("cp_sem"))
        cmp_sem = ctx.enter_context(nc.semaphore("cmp_sem"))
        # Per-slot DMA semaphores: concurrent DMAs may interleave their 16
        # per-engine increments, so a shared counter cannot signal which
        # DMA finished.  One sem per slot keeps at most one DMA in flight
        # per sem (enforced by the slot-reuse waits).
        in_sems = [ctx.enter_context(nc.semaphore(f"in_sem{k}"))
                   for k in range(SLOTS)]
        out_sems = [ctx.enter_context(nc.semaphore(f"out_sem{k}"))
                    for k in range(SLOTS)]
        block = ctx.enter_context(nc.Block())

        n_chunks = CMW // CHUNK    # 24
        cps = COLS // CHUNK        # mask chunks per sample: 3

        @block.sync
        def _(sync):
            for i in range(BPC):
                k, r = i % SLOTS, i // SLOTS
                if i >= SLOTS:
                    # slot reuse: out-DMA of tile i-SLOTS (same slot) done
                    sync.wait_ge(out_sems[k], 16 * r)
                src = img[512 * i:512 * (i + 1), :] \
                    .rearrange("(p q) m -> p (q m)", q=RPP)
                sync.dma_start(out=slots[k][:], in_=src) \
                    .then_inc(in_sems[k], 16)

        @block.tensor
        def _(tensor):
            tensor.wait_ge(const_sem, 32)   # cmfull + flags loaded
            for j in range(n_chunks):
                if j >= 4:
                    tensor.wait_ge(cp_sem, j - 3)   # PSUM bank reuse
                tensor.matmul(
                    psums[j % 4][:],
                    lhsT=cm_sb[:, :P],
                    rhs=cm_sb[:, P + j * CHUNK:P + (j + 1) * CHUNK],
                    start=True, stop=True,
                ).then_inc(pe_sem, 1)

        @block.vector
        def _(vector):
            for i in range(BPC):
                # copy this sample's mask chunks out of PSUM right before
                # its fused ops (pipelines the broadcast with the loop)
                for j in range(cps * i, cps * (i + 1)):
                    vector.wait_ge(pe_sem, j + 1)
                    vector.tensor_copy(
                        out=cmask[:, j * CHUNK:(j + 1) * CHUNK],
                        in_=psums[j % 4][:],
                    ).then_inc(cp_sem, 1)
                # DVE writes retire asynchronously even w.r.t. its own
                # later instructions - wait for this sample's mask copies
                # to land before the fused ops read them.
                vector.wait_ge(cp_sem, cps * (i + 1))
                k, r = i % SLOTS, i // SLOTS
                vector.wait_ge(in_sems[k], 16 * (r + 1))
                t = slots[k]
                for q in range(RPP):
                    ins = vector.scalar_tensor_tensor(
                        out=t[:, q * COLS:(q + 1) * COLS],
                        in0=cmask[:, i * COLS:(i + 1) * COLS],
                        scalar=flags_sb[:, RPP * i + q:RPP * i + q + 1],
                        in1=t[:, q * COLS:(q + 1) * COLS],
                        op0=mybir.AluOpType.max,
                        op1=mybir.AluOpType.mult,
                    )
                    if q == RPP - 1:
                        ins.then_inc(cmp_sem, 1)

        @block.scalar
        def _(scalar):
            scalar.dma_start(out=cm_sb[:], in_=cmfull[:]) \
                .then_inc(const_sem, 16)
            scalar.dma_start(out=flags_sb[:], in_=flags[:]) \
                .then_inc(const_sem, 16)
            for i in range(BPC):
                k = i % SLOTS
                scalar.wait_ge(cmp_sem, i + 1)
                src = slots[k][:]
                dst = out[512 * i:512 * (i + 1), :] \
                    .rearrange("(p q) m -> p (q m)", q=RPP)
                scalar.dma_start(out=dst, in_=src).then_inc(out_sems[k], 16)
            for k in range(SLOTS):
                n_k = len(range(k, BPC, SLOTS))
                scalar.wait_ge(out_sems[k], 16 * n_k)

    return nc


def _host_inputs(images, center_h, center_w, lo, hi):
    """Build (img [ROWS, COLS], flags [P, RPP*BPC], cmfull [1, P+CMW])
    for samples lo..hi-1."""
    hh = np.arange(H)
    ww = np.arange(W)
    img = np.ascontiguousarray(images[lo:hi].reshape(ROWS, COLS))
    flags = np.empty((P, RPP * BPC), dtype=np.float32)
    cmfull = np.ones((1, P + CMW), dtype=np.float32)
    for j, bb in enumerate(range(lo, hi)):
        ch = int(center_h[bb]); cw = int(center_w[bb])
        rkeep = (~((hh >= ch - HALF) & (hh < ch + HALF))).astype(np.float32)
        ckeep = (~((ww >= cw - HALF) & (ww < cw + HALF))).astype(np.float32)
        # flags[p, RPP*j + q] = rkeep[4p + q]
        flags[:, RPP * j:RPP * (j + 1)] = rkeep.reshape(P, RPP)
        cmfull[0, P + j * COLS:P + (j + 1) * COLS] = np.repeat(ckeep, C)
    return img, flags, cmfull


def kernel(images, labels, center_h, center_w):
    global _PROGRAM, LAST_RESULTS
    images = np.ascontiguousarray(np.asarray(images, dtype=np.float32))
    labels = np.asarray(labels, dtype=np.int32)
    center_h = np.asarray(center_h)
    center_w = np.asarray(center_w)
    assert images.shape == (B, H, W, C), images.shape

    if _PROGRAM is None:
        _PROGRAM = _build_program()

    in_maps = []
    for i in range(N_CORES):
        lo, hi = i * BPC, (i + 1) * BPC
        img, flags, cmfull = _host_inputs(images, center_h, center_w, lo, hi)
        in_maps.append({"img": img, "flags": flags, "cmfull": cmfull})

    trace = os.environ.get("CUTOUT_TRACE", "0") == "1"
    LAST_RESULTS = run_bass_kernel_spmd(
        _PROGRAM, in_maps, core_ids=list(range(N_CORES)), trace=trace,
    )
    out = np.concatenate(
        [LAST_RESULTS.results[i]["out"].reshape(BPC, H, W, C)
         for i in range(N_CORES)], axis=0)
    return out, labels


# revision 16
# speedup vs baseline: 1.1625x; 1.1625x over previous
"""CutOut kernel for Trainium2 (raw Bass), batch-sharded across 8 NeuronCores.

out[b,h,w,c] = 0 where (h,w) falls in the 50x50 rectangle centered at
(center_h[b], center_w[b]), else images[b,h,w,c]; labels pass through.

Strategy: data parallel over batch (8 samples/core, 24 MiB in + 24 MiB out
per core; the SBUF AXI fabric at ~436 GB/s is the binding resource ->
~110 us floor/core).  Written in raw Bass with explicit semaphores: the
walrus build in this environment accepts at most one inline sync-wait per
instruction, which rules out Tile's auto-generated multi-wait sync
(including its epilogue Drain) - standalone wait_ge instructions are the
legal form here.

Per core:
  consts: on the ACT HWDGE ring (idle at start, so image DMAs on the SP
          ring begin immediately): cmfull [1, 128+8*1536] (128 ones then
          per-sample column keep-masks) and flags [128, 32] ("row outside
          cutout?" 0/1 per row, laid out so tile i / subrow q reads column
          4i+q as a per-partition scalar).
  init:   24 K=1 matmuls (ones-weights outer product) broadcast the column
          masks to a [128, 12288] SBUF constant via PSUM; each sample's 3
          chunks are copied out right before that sample's compute, so the
          broadcast pipelines with the main loop instead of serializing.
  loop:   one 3 MiB DMA per sample (512 rows packed 4-per-partition,
          24 KiB contiguous per partition), four fused DVE ops
              out = (colmask max rowflag[p]) * img
          (exact 0/1 values -> bit-exact), one 3 MiB DMA out on ACT's ring.

Host prepares only O(B*(H+W)) flag values from the centers; all O(B*H*W)
work happens on device.
"""

import os
import sys
from contextlib import ExitStack

import numpy as np

if "/opt/trn_rl_repo" not in sys.path:
    sys.path.insert(0, "/opt/trn_rl_repo")

import concourse.bass as bass
import concourse.mybir as mybir
from concourse.bass_utils import run_bass_kernel_spmd

B, H, W, C = 64, 512, 512, 3
N_CORES = 8
BPC = B // N_CORES                 # samples per core = tiles per core
HALF = 25                          # 50x50 patch
ROWS = BPC * H                     # 4096 rows per core
COLS = W * C                       # 1536 floats per row
RPP = 4                            # rows packed per partition (512/128)
P = 128                            # SBUF partitions
CHUNK = 512                        # PSUM bank width (f32)
CMW = BPC * COLS                   # 12288 column-mask width
SLOTS = 4                          # in-flight sample tiles

F32 = mybir.dt.float32

_PROGRAM = None
LAST_RESULTS = None                # BassKernelResults of the latest run


def _build_program():
    nc = bass.Bass("TRN2", target_bir_lowering=False, debug=False,
                   num_devices=N_CORES)
    img = nc.dram_tensor("img", [ROWS, COLS], F32, kind="ExternalInput").ap()
    flags = nc.dram_tensor("flags", [P, RPP * BPC], F32,
                           kind="ExternalInput").ap()
    cmfull = nc.dram_tensor("cmfull", [1, P + CMW], F32,
                            kind="ExternalInput").ap()
    out = nc.dram_tensor("out", [ROWS, COLS], F32, kind="ExternalOutput").ap()

    with ExitStack() as ctx:
        slots = [ctx.enter_context(
                     nc.sbuf_tensor(f"slot{k}", [P, RPP * COLS], F32))
                 for k in range(SLOTS)]
        flags_sb = ctx.enter_context(
            nc.sbuf_tensor("flags_sb", [P, RPP * BPC], F32))
        cm_sb = ctx.enter_context(nc.sbuf_tensor("cm_sb", [1, P + CMW], F32))
        cmask = ctx.enter_context(nc.sbuf_tensor("cmask", [P, CMW], F32))
        psums = [ctx.enter_context(nc.psum_tensor(f"ps{k}", [P, CHUNK], F32))
                 for k in range(4)]
        const_sem = ctx.enter_context(nc.semaphore("const_sem"))
        pe_sem = ctx.enter_context(nc.semaphore("pe_sem"))
        cp_sem = ctx.enter_context(nc.semaphore("cp_sem"))
        cmp_sem = ctx.enter_context(nc.semaphore("cmp_sem"))
        # Per-slot DMA semaphores: concurrent DMAs may interleave their 16
        # per-engine increments, so a shared counter cannot signal which
        # DMA finished.  One sem per slot keeps at most one DMA in flight
        # per sem (enforced by the slot-reuse waits).
        in_sems = [ctx.enter_context(nc.semaphore(f"in_sem{k}"))
                   for k in range(SLOTS)]
        out_sems = [ctx.enter_context(nc.semaphore(f"out_sem{k}"))
                    for k in range(SLOTS)]
        block = ctx.enter_context(nc.Block())

        n_chunks = CMW // CHUNK    # 24
        cps = COLS // CHUNK        # mask chunks per sample: 3

        @block.sync
        def _(sync):
            for i in range(BPC):
                k, r = i % SLOTS, i // SLOTS
                if i >= SLOTS:
                    # slot reuse: out-DMA of tile i-SLOTS (same slot) done
                    sync.wait_ge(out_sems[k], 16 * r)
                src = img[512 * i:512 * (i + 1), :] \
                    .rearrange("(p q) m -> p q m", q=RPP)
                dst = slots[k][:].rearrange("p (q m) -> p q m", q=RPP)
                sync.dma_start(out=dst, in_=src) \
                    .then_inc(in_sems[k], 16)

        @block.tensor
        def _(tensor):
            tensor.wait_ge(const_sem, 32)   # cmfull + flags loaded
            for j in range(n_chunks):
                if j >= 4:
                    tensor.wait_ge(cp_sem, j - 3)   # PSUM bank reuse
                tensor.matmul(
                    psums[j % 4][:],
                    lhsT=cm_sb[:, :P],
                    rhs=cm_sb[:, P + j * CHUNK:P + (j + 1) * CHUNK],
                    start=True, stop=True,
                ).then_inc(pe_sem, 1)

        @block.vector
        def _(vector):
            for i in range(BPC):
                # copy this sample's mask chunks out of PSUM right before
                # its fused ops (pipelines the broadcast with the loop)
                for j in range(cps * i, cps * (i + 1)):
                    vector.wait_ge(pe_sem, j + 1)
                    vector.tensor_copy(
                        out=cmask[:, j * CHUNK:(j + 1) * CHUNK],
                        in_=psums[j % 4][:],
                    ).then_inc(cp_sem, 1)
                # DVE writes retire asynchronously even w.r.t. its own
                # later instructions - wait for this sample's mask copies
                # to land before the fused ops read them.
                vector.wait_ge(cp_sem, cps * (i + 1))
                k, r = i % SLOTS, i // SLOTS
                vector.wait_ge(in_sems[k], 16 * (r + 1))
                t = slots[k]
                for q in range(RPP):
                    ins = vector.scalar_tensor_tensor(
                        out=t[:, q * COLS:(q + 1) * COLS],
                        in0=cmask[:, i * COLS:(i + 1) * COLS],
                        scalar=flags_sb[:, RPP * i + q:RPP * i + q + 1],
                        in1=t[:, q * COLS:(q + 1) * COLS],
                        op0=mybir.AluOpType.max,
                        op1=mybir.AluOpType.mult,
                    )
                    if q == RPP - 1:
                        ins.then_inc(cmp_sem, 1)

        @block.scalar
        def _(scalar):
            scalar.dma_start(out=cm_sb[:], in_=cmfull[:]) \
                .then_inc(const_sem, 16)
            scalar.dma_start(out=flags_sb[:], in_=flags[:]) \
                .then_inc(const_sem, 16)
            for i in range(BPC):
                k = i % SLOTS
                scalar.wait_ge(cmp_sem, i + 1)
                src = slots[k][:].rearrange("p (q m) -> p q m", q=RPP)
                dst = out[512 * i:512 * (i + 1), :] \
                    .rearrange("(p q) m -> p q m", q=RPP)
                scalar.dma_start(out=dst, in_=src).then_inc(out_sems[k], 16)
            for k in range(SLOTS):
                n_k = len(range(k, BPC, SLOTS))
                scalar.wait_ge(out_sems[k], 16 * n_k)

    return nc


def _host_inputs(images, center_h, center_w, lo, hi):
    """Build (img [ROWS, COLS], flags [P, RPP*BPC], cmfull [1, P+CMW])
    for samples lo..hi-1."""
    hh = np.arange(H)
    ww = np.arange(W)
    img = np.ascontiguousarray(images[lo:hi].reshape(ROWS, COLS))
    flags = np.empty((P, RPP * BPC), dtype=np.float32)
    cmfull = np.ones((1, P + CMW), dtype=np.float32)
    for j, bb in enumerate(range(lo, hi)):
        ch = int(center_h[bb]); cw = int(center_w[bb])
        rkeep = (~((hh >= ch - HALF) & (hh < ch + HALF))).astype(np.float32)
        ckeep = (~((ww >= cw - HALF) & (ww < cw + HALF))).astype(np.float32)
        # flags[p, RPP*j + q] = rkeep[4p + q]
        flags[:, RPP * j:RPP * (j + 1)] = rkeep.reshape(P, RPP)
        cmfull[0, P + j * COLS:P + (j + 1) * COLS] = np.repeat(ckeep, C)
    return img, flags, cmfull


def kernel(images, labels, center_h, center_w):
    global _PROGRAM, LAST_RESULTS
    images = np.ascontiguousarray(np.asarray(images, dtype=np.float32))
    labels = np.asarray(labels, dtype=np.int32)
    center_h = np.asarray(center_h)
    center_w = np.asarray(center_w)
    assert images.shape == (B, H, W, C), images.shape

    if _PROGRAM is None:
        _PROGRAM = _build_program()

    in_maps = []
    for i in range(N_CORES):
        lo, hi = i * BPC, (i + 1) * BPC
        img, flags, cmfull = _host_inputs(images, center_h, center_w, lo, hi)
        in_maps.append({"img": img, "flags": flags, "cmfull": cmfull})

    trace = os.environ.get("CUTOUT_TRACE", "0") == "1"
    LAST_RESULTS = run_bass_kernel_spmd(
        _PROGRAM, in_maps, core_ids=list(range(N_CORES)), trace=trace,
    )
    out = np.concatenate(
        [LAST_RESULTS.results[i]["out"].reshape(BPC, H, W, C)
         for i in range(N_CORES)], axis=0)
    return out, labels


# revision 17
# speedup vs baseline: 1.1660x; 1.0030x over previous
"""CutOut kernel for Trainium2 (raw Bass), batch-sharded across 8 NeuronCores.

out[b,h,w,c] = 0 where (h,w) falls in the 50x50 rectangle centered at
(center_h[b], center_w[b]), else images[b,h,w,c]; labels pass through.

Strategy: data parallel over batch (8 samples/core, 24 MiB in + 24 MiB out
per core; the SBUF AXI fabric at ~436 GB/s is the binding resource ->
~110 us floor/core).  Written in raw Bass with explicit semaphores: the
walrus build in this environment accepts at most one inline sync-wait per
instruction, which rules out Tile's auto-generated multi-wait sync
(including its epilogue Drain) - standalone wait_ge instructions are the
legal form here.

Per core:
  consts: on the ACT HWDGE ring (idle at start, so image DMAs on the SP
          ring begin immediately): cmfull [1, 128+8*1536] (128 ones then
          per-sample column keep-masks) and flags [128, 32] ("row outside
          cutout?" 0/1 per row, laid out so tile i / subrow q reads column
          4i+q as a per-partition scalar).
  init:   24 K=1 matmuls (ones-weights outer product) broadcast the column
          masks to a [128, 12288] SBUF constant via PSUM; each sample's 3
          chunks are copied out right before that sample's compute, so the
          broadcast pipelines with the main loop instead of serializing.
  loop:   one 3 MiB DMA per sample (512 rows packed 4-per-partition,
          24 KiB contiguous per partition), four fused DVE ops
              out = (colmask max rowflag[p]) * img
          (exact 0/1 values -> bit-exact), one 3 MiB DMA out on ACT's ring.

Host prepares only O(B*(H+W)) flag values from the centers; all O(B*H*W)
work happens on device.
"""

import os
import sys
from contextlib import ExitStack

import numpy as np

if "/opt/trn_rl_repo" not in sys.path:
    sys.path.insert(0, "/opt/trn_rl_repo")

import concourse.bass as bass
import concourse.mybir as mybir
from concourse.bass_utils import run_bass_kernel_spmd

B, H, W, C = 64, 512, 512, 3
N_CORES = 8
BPC = B // N_CORES                 # samples per core = tiles per core
HALF = 25                          # 50x50 patch
ROWS = BPC * H                     # 4096 rows per core
COLS = W * C                       # 1536 floats per row
RPP = 4                            # rows packed per partition (512/128)
P = 128                            # SBUF partitions
CHUNK = 512                        # PSUM bank width (f32)
CMW = BPC * COLS                   # 12288 column-mask width
SLOTS = 4                          # in-flight sample tiles

F32 = mybir.dt.float32

_PROGRAM = None
LAST_RESULTS = None                # BassKernelResults of the latest run


def _build_program():
    nc = bass.Bass("TRN2", target_bir_lowering=False, debug=False,
                   num_devices=N_CORES)
    img = nc.dram_tensor("img", [ROWS, COLS], F32, kind="ExternalInput").ap()
    flags = nc.dram_tensor("flags", [P, RPP * BPC], F32,
                           kind="ExternalInput").ap()
    cmfull = nc.dram_tensor("cmfull", [1, P + CMW], F32,
                            kind="ExternalInput").ap()
    out = nc.dram_tensor("out", [ROWS, COLS], F32, kind="ExternalOutput").ap()

    with ExitStack() as ctx:
        slots = [ctx.enter_context(
                     nc.sbuf_tensor(f"slot{k}", [P, RPP * COLS], F32))
                 for k in range(SLOTS)]
        flags_sb = ctx.enter_context(
            nc.sbuf_tensor("flags_sb", [P, RPP * BPC], F32))
        cm_sb = ctx.enter_context(nc.sbuf_tensor("cm_sb", [1, P + CMW], F32))
        cmask = ctx.enter_context(nc.sbuf_tensor("cmask", [P, CMW], F32))
        psums = [ctx.enter_context(nc.psum_tensor(f"ps{k}", [P, CHUNK], F32))
                 for k in range(4)]
        const_sem = ctx.enter_context(nc.semaphore("const_sem"))
        pe_sem = ctx.enter_context(nc.semaphore("pe_sem"))
        cp_sem = ctx.enter_context(nc.semaphore("cp_sem"))
        cmp_sem = ctx.enter_context(nc.semaphore("cmp_sem"))
        # Per-slot DMA semaphores: concurrent DMAs may interleave their 16
        # per-engine increments, so a shared counter cannot signal which
        # DMA finished.  One sem per slot keeps at most one DMA in flight
        # per sem (enforced by the slot-reuse waits).
        in_sems = [ctx.enter_context(nc.semaphore(f"in_sem{k}"))
                   for k in range(SLOTS)]
        out_sems = [ctx.enter_context(nc.semaphore(f"out_sem{k}"))
                    for k in range(SLOTS)]
        block = ctx.enter_context(nc.Block())

        n_chunks = CMW // CHUNK    # 24
        cps = COLS // CHUNK        # mask chunks per sample: 3

        @block.sync
        def _(sync):
            for i in range(BPC):
                k, r = i % SLOTS, i // SLOTS
                if i >= SLOTS:
                    # slot reuse: out-DMA of tile i-SLOTS (same slot) done
                    sync.wait_ge(out_sems[k], 16 * r)
                src = img[512 * i:512 * (i + 1), :] \
                    .rearrange("(p q) m -> p (q m)", q=RPP) \
                    .rearrange("p (s n) -> p s n", s=6)
                dst = slots[k][:].rearrange("p (s n) -> p s n", s=6)
                sync.dma_start(out=dst, in_=src) \
                    .then_inc(in_sems[k], 16)

        @block.tensor
        def _(tensor):
            tensor.wait_ge(const_sem, 32)   # cmfull + flags loaded
            for j in range(n_chunks):
                if j >= 4:
                    tensor.wait_ge(cp_sem, j - 3)   # PSUM bank reuse
                tensor.matmul(
                    psums[j % 4][:],
                    lhsT=cm_sb[:, :P],
                    rhs=cm_sb[:, P + j * CHUNK:P + (j + 1) * CHUNK],
                    start=True, stop=True,
                ).then_inc(pe_sem, 1)

        @block.vector
        def _(vector):
            for i in range(BPC):
                # copy this sample's mask chunks out of PSUM right before
                # its fused ops (pipelines the broadcast with the loop)
                for j in range(cps * i, cps * (i + 1)):
                    vector.wait_ge(pe_sem, j + 1)
                    vector.tensor_copy(
                        out=cmask[:, j * CHUNK:(j + 1) * CHUNK],
                        in_=psums[j % 4][:],
                    ).then_inc(cp_sem, 1)
                # DVE writes retire asynchronously even w.r.t. its own
                # later instructions - wait for this sample's mask copies
                # to land before the fused ops read them.
                vector.wait_ge(cp_sem, cps * (i + 1))
                k, r = i % SLOTS, i // SLOTS
                vector.wait_ge(in_sems[k], 16 * (r + 1))
                t = slots[k]
                for q in range(RPP):
                    ins = vector.scalar_tensor_tensor(
                        out=t[:, q * COLS:(q + 1) * COLS],
                        in0=cmask[:, i * COLS:(i + 1) * COLS],
                        scalar=flags_sb[:, RPP * i + q:RPP * i + q + 1],
                        in1=t[:, q * COLS:(q + 1) * COLS],
                        op0=mybir.AluOpType.max,
                        op1=mybir.AluOpType.mult,
                    )
                    if q == RPP - 1:
                        ins.then_inc(cmp_sem, 1)

        @block.scalar
        def _(scalar):
            scalar.dma_start(out=cm_sb[:], in_=cmfull[:]) \
                .then_inc(const_sem, 16)
            scalar.dma_start(out=flags_sb[:], in_=flags[:]) \
                .then_inc(const_sem, 16)
            for i in range(BPC):
                k = i % SLOTS
                scalar.wait_ge(cmp_sem, i + 1)
                src = slots[k][:].rearrange("p (s n) -> p s n", s=6)
                dst = out[512 * i:512 * (i + 1), :] \
                    .rearrange("(p q) m -> p (q m)", q=RPP) \
                    .rearrange("p (s n) -> p s n", s=6)
                scalar.dma_start(out=dst, in_=src).then_inc(out_sems[k], 16)
            for k in range(SLOTS):
                n_k = len(range(k, BPC, SLOTS))
                scalar.wait_ge(out_sems[k], 16 * n_k)

    return nc


def _host_inputs(images, center_h, center_w, lo, hi):
    """Build (img [ROWS, COLS], flags [P, RPP*BPC], cmfull [1, P+CMW])
    for samples lo..hi-1."""
    hh = np.arange(H)
    ww = np.arange(W)
    img = np.ascontiguousarray(images[lo:hi].reshape(ROWS, COLS))
    flags = np.empty((P, RPP * BPC), dtype=np.float32)
    cmfull = np.ones((1, P + CMW), dtype=np.float32)
    for j, bb in enumerate(range(lo, hi)):
        ch = int(center_h[bb]); cw = int(center_w[bb])
        rkeep = (~((hh >= ch - HALF) & (hh < ch + HALF))).astype(np.float32)
        ckeep = (~((ww >= cw - HALF) & (ww < cw + HALF))).astype(np.float32)
        # flags[p, RPP*j + q] = rkeep[4p + q]
        flags[:, RPP * j:RPP * (j + 1)] = rkeep.reshape(P, RPP)
        cmfull[0, P + j * COLS:P + (j + 1) * COLS] = np.repeat(ckeep, C)
    return img, flags, cmfull


def kernel(images, labels, center_h, center_w):
    global _PROGRAM, LAST_RESULTS
    images = np.ascontiguousarray(np.asarray(images, dtype=np.float32))
    labels = np.asarray(labels, dtype=np.int32)
    center_h = np.asarray(center_h)
    center_w = np.asarray(center_w)
    assert images.shape == (B, H, W, C), images.shape

    if _PROGRAM is None:
        _PROGRAM = _build_program()

    in_maps = []
    for i in range(N_CORES):
        lo, hi = i * BPC, (i + 1) * BPC
        img, flags, cmfull = _host_inputs(images, center_h, center_w, lo, hi)
        in_maps.append({"img": img, "flags": flags, "cmfull": cmfull})

    trace = os.environ.get("CUTOUT_TRACE", "0") == "1"
    LAST_RESULTS = run_bass_kernel_spmd(
        _PROGRAM, in_maps, core_ids=list(range(N_CORES)), trace=trace,
    )
    out = np.concatenate(
        [LAST_RESULTS.results[i]["out"].reshape(BPC, H, W, C)
         for i in range(N_CORES)], axis=0)
    return out, labels
